# revision 26
# baseline (speedup 1.0000x reference)
"""Trainium2 Bass kernel for joint node+edge graph attention.

Problem: nn_Attention_71975061946902
  B=2, N=1024 nodes (dim 128), E=2048 edges (dim 256), L=N+E=3072,
  HEADS=8, DHEAD=64, INNER=512. Full attention over the joint sequence
  with a pair mask (mask_i & mask_j), per-type in/out projections.

Sharding: 16 (batch, head) slices -> 8 cores; each core takes one batch
and two adjacent heads. Host compacts the sequence to mask-valid rows
(~half), transposes inputs to contraction-major layout, slices weights
per head pair, and folds SCALE into Wq/bq (exact: SCALE = 2^-3).

Device: scores are computed transposed (ST[j,i]) so the pair mask is
applied for free inside the softmax exp via per-partition scale/bias on
the Activation instruction: exp(m_j * S + (1-m_j)*(-30000)). No row-max
subtraction is needed (|scores| < ~4). A ones column appended to V makes
the same PV matmul produce the softmax denominator. Masked query rows
(uniform attention over all keys) are reconstructed exactly on the host
and scattered into the output. All inputs arrive in two packed blobs
(one DMA each); outputs leave in one packed blob (two DMAs).
"""

import sys

for _p in ("/opt/trn_rl_repo",):
    if _p not in sys.path:
        sys.path.insert(0, _p)

import numpy as np

B = 2
N = 1024
E = 2048
DIM = 128
EDGE_DIM = 256
HEADS = 8
DHEAD = 64
INNER = HEADS * DHEAD
SCALE = DHEAD ** -0.5  # 0.125, exact power of two
NEGC = -30000.0
L = N + E

COMPACT = True  # compact the sequence to mask-valid rows before launch
FPR_ENABLE = True  # float32r matmuls (4x PE throughput, ~1e-4 rel err)

_NC_CACHE: dict = {}
LAST_RES = None  # BassKernelResults of the most recent launch (for test.py)
LAST_WALL_NS = None


def _wpack_layout(Jt):
    """Column layout of the packed weight/bias/mask blob [128, WC]."""
    off = {}
    c = 0
    for name, w in (("wq_n", 128), ("wk_n", 128), ("wv_n", 128),
                    ("wq_e0", 128), ("wq_e1", 128),
                    ("wk_e0", 128), ("wk_e1", 128),
                    ("wv_e0", 128), ("wv_e1", 128),
                    ("wo", 128), ("weo", 256),
                    ("bq", 1), ("bk", 1), ("beq", 1), ("bek", 1),
                    ("mT", Jt), ("negC", Jt),
                    ("bvr", 128), ("bevr", 128)):
        off[name] = (c, c + w)
        c += w
    return off, c


def build_nc(Jn: int, Je: int):
    """Build the SPMD Bass program for one core: one batch, two heads."""
    from contextlib import ExitStack

    import concourse.tile as tile
    from concourse import bacc, mybir
    from concourse.bass import MemorySpace, ts

    FP = mybir.dt.float32
    FPR = mybir.dt.float32r if FPR_ENABLE else mybir.dt.float32
    ACT = mybir.ActivationFunctionType

    Nn, Ne = 128 * Jn, 128 * Je
    Jt = Jn + Je
    Lc = 128 * Jt
    WOFF, WC = _wpack_layout(Jt)
    XC = Nn + 2 * Ne
    OCN, OC = 128 * Jn, 128 * Jn + 256 * Je

    nc = bacc.Bacc("TRN2", target_bir_lowering=False)

    xpack = nc.dram_tensor("xpack", [DIM, XC], FPR, kind="ExternalInput")
    wpack = nc.dram_tensor("wpack", [128, WC], FPR, kind="ExternalInput")
    opack = nc.dram_tensor("opack", [128, OC], FP, kind="ExternalOutput")

    with tile.TileContext(nc) as tc, ExitStack() as ctx:
        singles = ctx.enter_context(tc.tile_pool(name="singles", bufs=1))

        s_x = singles.tile([DIM, XC], FPR)
        nc.sync.dma_start(s_x[:, :], xpack[:, :])
        s_w = singles.tile([128, WC], FPR)
        nc.sync.dma_start(s_w[:, :], wpack[:, :])

        def W(name):
            a, b = WOFF[name]
            return s_w[:, a:b]

        def Wf(name):
            return W(name).bitcast(FP)

        x_n = s_x[:, 0:Nn]
        x_e = [s_x[:, Nn + c * Ne:Nn + (c + 1) * Ne] for c in range(2)]

        # 512-wide segments of the compacted axis (node region then edges)
        segs = []
        for base, nblk in ((0, Nn), (Nn, Ne)):
            for blk0 in range(0, nblk, 512):
                segs.append((base + blk0, min(512, nblk - blk0)))
        jmap = []  # jtile t -> (segment index, column offset)
        for t in range(Jt):
            j0 = 128 * t
            for k, (sb, sw) in enumerate(segs):
                if sb <= j0 < sb + sw:
                    jmap.append((k, j0 - sb))
                    break

        # per-segment Q/K tiles and per-jtile V tiles: fine-grained deps let
        # attention start as soon as its own slices are projected
        q_blks = [singles.tile([128, sw], FPR, name=f"qb{k}")
                  for k, (sb, sw) in enumerate(segs)]
        k_blks = [singles.tile([128, sw], FPR, name=f"kb{k}")
                  for k, (sb, sw) in enumerate(segs)]
        v_t = [singles.tile([128, 2, 65], FPR, name=f"v{t}")
               for t in range(Jt)]
        s_oT = singles.tile([128, Lc], FPR)
        s_out = singles.tile([128, OC], FP)
        s_bvb = singles.tile([128, 128], FPR)
        s_bevb = singles.tile([128, 128], FPR)

        for t in range(Jt):
            nc.vector.memset(v_t[t][:, :, 64:65].bitcast(FP), 1.0)
        a0, _ = WOFF["bvr"]
        nc.gpsimd.partition_broadcast(s_bvb[:, :], s_w[0:1, a0:a0 + 128],
                                      channels=128)
        a0, _ = WOFF["bevr"]
        nc.gpsimd.partition_broadcast(s_bevb[:, :], s_w[0:1, a0:a0 + 128],
                                      channels=128)

        # ---- projections ----
        with tc.tile_pool(name="pps", bufs=3, space=MemorySpace.PSUM) as pps:
            for k, (sb, sw) in enumerate(segs):
                is_node = sb < Nn
                if is_node:
                    xs = [x_n[:, sb:sb + sw]]
                    wq_t, wk_t = [W("wq_n")], [W("wk_n")]
                    bq_t, bk_t = Wf("bq"), Wf("bk")
                else:
                    xs = [xe[:, sb - Nn:sb - Nn + sw] for xe in x_e]
                    wq_t = [W("wq_e0"), W("wq_e1")]
                    wk_t = [W("wk_e0"), W("wk_e1")]
                    bq_t, bk_t = Wf("beq"), Wf("bek")
                nchunk = len(xs)
                for wt, bias_t, dst in ((wk_t, bk_t, k_blks[k]),
                                        (wq_t, bq_t, q_blks[k])):
                    ps = pps.tile([128, 512], FP, tag="proj")
                    for c in range(nchunk):
                        nc.tensor.matmul(ps[:, :sw], (wt[c]), (xs[c]),
                                         start=(c == 0),
                                         stop=(c == nchunk - 1))
                    nc.vector.tensor_scalar_add(dst[:, :sw], ps[:, :sw],
                                                bias_t)

            for t in range(Jt):
                ps = pps.tile([128, 128], FP, tag="projv")
                if t < Jn:
                    nc.tensor.matmul(ps[:, :], (x_n[:, ts(t, 128)]),
                                     (W("wv_n")), start=True, stop=True)
                else:
                    te = t - Jn
                    nc.tensor.matmul(ps[:, :], (x_e[0][:, ts(te, 128)]),
                                     (W("wv_e0")), start=True, stop=False)
                    nc.tensor.matmul(ps[:, :], (x_e[1][:, ts(te, 128)]),
                                     (W("wv_e1")), start=False, stop=True)
                vb = s_bvb if t < Jn else s_bevb
                nc.vector.tensor_add(
                    v_t[t][:, :, 0:64],
                    ps.rearrange("p (h d) -> p h d", h=2),
                    vb.rearrange("p (h d) -> p h d", h=2),
                )

        # ---- attention ----
        s_mT, s_negC = Wf("mT"), Wf("negC")
        with tc.tile_pool(name="stps", bufs=2, space=MemorySpace.PSUM) as stps, \
                tc.tile_pool(name="pvps", bufs=2, space=MemorySpace.PSUM) as pvps, \
                tc.tile_pool(name="ppool", bufs=4) as ppool, \
                tc.tile_pool(name="npool", bufs=4) as npool:
            for kq, (i0, w) in enumerate(segs):
                po = [pvps.tile([65, 512], FP, tag="pv0", name="po0"),
                      pvps.tile([65, 512], FP, tag="pv1", name="po1")]
                for t in range(Jt):
                    kk, joff = jmap[t]
                    st = stps.tile([128, 2, 512], FP, tag="st")
                    for h in range(2):
                        nc.tensor.matmul(st[:, h, :w],
                                         (k_blks[kk][64 * h:64 * h + 64,
                                                     joff:joff + 128]),
                                         (q_blks[kq][64 * h:64 * h + 64, :w]),
                                         start=True, stop=True)
                    pt = ppool.tile([128, 2, 512], FPR, tag="p")
                    # exp with fused pair mask: exp(m_j * S + (1-m_j)*NEGC)
                    nc.scalar.activation(pt[:, :, :w], st[:, :, :w], ACT.Exp,
                                         bias=s_negC[:, t:t + 1],
                                         scale=s_mT[:, t:t + 1])
                    for h in range(2):
                        nc.tensor.matmul(po[h][:, :w], (v_t[t][:, h, :]),
                                         (pt[:, h, :w]),
                                         start=(t == 0), stop=(t == Jt - 1))
                # normalize: rows 0:64 are sum(P*v), row 64 is sum(P)
                for h in range(2):
                    dsb = npool.tile([1, 512], FP, tag="dsb")
                    nc.vector.reciprocal(dsb[:, :w], po[h][64:65, :w])
                    rb = npool.tile([64, 512], FP, tag="rb")
                    nc.gpsimd.partition_broadcast(rb[:, :w], dsb[0:1, :w],
                                                  channels=64)
                    nc.vector.tensor_mul(s_oT[64 * h:64 * h + 64, i0:i0 + w],
                                         po[h][0:64, :w], rb[:, :w])

        # ---- output projections ----
        with tc.tile_pool(name="ops", bufs=2, space=MemorySpace.PSUM) as ops:
            for t in range(Jt):
                if t < Jn:
                    ps = ops.tile([128, DIM], FP, tag="opn")
                    nc.tensor.matmul(ps[:, :], (s_oT[:, ts(t, 128)]),
                                     (W("wo")), start=True, stop=True)
                    nc.vector.tensor_copy(s_out[:, ts(t, 128)], ps[:, :])
                else:
                    te = t - Jn
                    ps = ops.tile([128, EDGE_DIM], FP, tag="ope")
                    nc.tensor.matmul(ps[:, :], (s_oT[:, ts(t, 128)]),
                                     (W("weo")), start=True, stop=True)
                    nc.vector.tensor_copy(s_out[:, OCN + te * 256:
                                                OCN + (te + 1) * 256], ps[:, :])
        nc.sync.dma_start(opack[:, 0:OCN], s_out[:, 0:OCN])
        nc.sync.dma_start(opack[:, OCN:OC], s_out[:, OCN:OC])

    nc.compile()
    return nc


def _get_nc(Jn: int, Je: int):
    key = (Jn, Je, FPR_ENABLE)
    if key not in _NC_CACHE:
        _NC_CACHE[key] = build_nc(Jn, Je)
    return _NC_CACHE[key]


def _prep_host(nodes, edges, mask):
    """Per-batch compaction and layout prep."""
    per_b = []
    if COMPACT:
        idx_n = [np.nonzero(mask[b, :N])[0] for b in range(B)]
        idx_e = [np.nonzero(mask[b, N:])[0] for b in range(B)]
    else:
        idx_n = [np.arange(N)] * B
        idx_e = [np.arange(E)] * B
    Jn = max(1, max((len(i) + 127) // 128 for i in idx_n))
    Je = max(1, max((len(i) + 127) // 128 for i in idx_e))
    Nn, Ne = 128 * Jn, 128 * Je
    Jt = Jn + Je
    Lc = Nn + Ne
    for b in range(B):
        ln, le = len(idx_n[b]), len(idx_e[b])
        xp = np.zeros((DIM, Nn + 2 * Ne), np.float32)
        if ln:
            xp[:, :ln] = nodes[b][idx_n[b]].T
        if le:
            eT = edges[b][idx_e[b]].T  # [256, le]
            xp[:, Nn:Nn + le] = eT[:128]
            xp[:, Nn + Ne:Nn + Ne + le] = eT[128:]
        mc = np.zeros(Lc, np.float32)
        if ln:
            mc[:ln] = mask[b, :N][idx_n[b]].astype(np.float32)
        if le:
            mc[Nn:Nn + le] = mask[b, N:][idx_e[b]].astype(np.float32)
        mT = np.ascontiguousarray(mc.reshape(Jt, 128).T)
        negC = np.ascontiguousarray(((1.0 - mc) * NEGC).reshape(Jt, 128).T)
        per_b.append(dict(xpack=np.ascontiguousarray(xp), mT=mT, negC=negC))
    return per_b, idx_n, idx_e, Jn, Je


def _core_weights(inputs, h0):
    """Unpacked per-core weight dict (heads h0, h0+1)."""
    (Wq, bq, Wk, bk, Wv, bv, Weq, beq, Wek, bek, Wev, bev,
     Wo, bo, Weo, beo) = inputs
    sl = slice(DHEAD * h0, DHEAD * h0 + 128)
    sc = np.float32(SCALE)
    m = {}
    m["wq_n"] = Wq[:, sl] * sc
    m["wk_n"] = Wk[:, sl]
    m["wv_n"] = Wv[:, sl]
    m["wq_e"] = (Weq[:, sl] * sc).reshape(2, 128, 128)
    m["wk_e"] = Wek[:, sl].reshape(2, 128, 128)
    m["wv_e"] = Wev[:, sl].reshape(2, 128, 128)
    m["wo"] = Wo[sl, :]
    m["weo"] = Weo[sl, :]
    m["bq2"] = (bq[sl] * sc).reshape(128, 1)
    m["bk2"] = bk[sl].reshape(128, 1)
    m["bvr"] = bv[sl].reshape(1, 128)
    m["beq2"] = (beq[sl] * sc).reshape(128, 1)
    m["bek2"] = bek[sl].reshape(128, 1)
    m["bevr"] = bev[sl].reshape(1, 128)
    return m


def _pack_w(m, mT, negC, Jt):
    """Pack per-core weights+masks into the wpack blob [128, WC]."""
    WOFF, WC = _wpack_layout(Jt)
    w = np.zeros((128, WC), np.float32)

    def put(name, arr):
        a, b = WOFF[name]
        w[:arr.shape[0], a:b] = arr

    put("wq_n", m["wq_n"]); put("wk_n", m["wk_n"]); put("wv_n", m["wv_n"])
    put("wq_e0", m["wq_e"][0]); put("wq_e1", m["wq_e"][1])
    put("wk_e0", m["wk_e"][0]); put("wk_e1", m["wk_e"][1])
    put("wv_e0", m["wv_e"][0]); put("wv_e1", m["wv_e"][1])
    put("wo", m["wo"]); put("weo", m["weo"])
    put("bq", m["bq2"]); put("bk", m["bk2"])
    put("beq", m["beq2"]); put("bek", m["bek2"])
    put("mT", mT); put("negC", negC)
    put("bvr", m["bvr"]); put("bevr", m["bevr"])
    return np.ascontiguousarray(w)


def kernel(nodes, edges, mask, Wq, bq, Wk, bk, Wv, bv,
           Weq, beq, Wek, bek, Wev, bev, Wo, bo, Weo, beo):
    from concourse.bass_utils import run_bass_kernel_spmd

    nodes = np.asarray(nodes, np.float32)
    edges = np.asarray(edges, np.float32)
    mask = np.asarray(mask).astype(bool)
    ws = tuple(np.asarray(a, np.float32) for a in
               (Wq, bq, Wk, bk, Wv, bv, Weq, beq, Wek, bek, Wev, bev,
                Wo, bo, Weo, beo))
    (Wq, bq, Wk, bk, Wv, bv, Weq, beq, Wek, bek, Wev, bev,
     Wo, bo, Weo, beo) = ws

    per_b, idx_n, idx_e, Jn, Je = _prep_host(nodes, edges, mask)
    Jt = Jn + Je
    nc = _get_nc(Jn, Je)

    in_maps = []
    for core in range(8):
        b = core // 4
        h0 = (core % 4) * 2
        cw = _core_weights(ws, h0)
        in_maps.append({
            "xpack": per_b[b]["xpack"],
            "wpack": _pack_w(cw, per_b[b]["mT"], per_b[b]["negC"], Jt),
        })

    import time as _time

    _t0 = _time.perf_counter()
    res = run_bass_kernel_spmd(nc, in_maps, core_ids=list(range(8)))
    global LAST_RES, LAST_WALL_NS
    LAST_WALL_NS = (_time.perf_counter() - _t0) * 1e9
    LAST_RES = res
    results = res.results

    OCN = 128 * Jn
    node_out = np.empty((B, N, DIM), np.float32)
    edge_out = np.empty((B, E, EDGE_DIM), np.float32)
    for b in range(B):
        acc = results[4 * b]["opack"].copy()
        for k in range(1, 4):
            acc += results[4 * b + k]["opack"]
        # unpack: node tile t at cols [t*128,(t+1)*128) with rows=i, cols=o
        acc_n = acc[:, :OCN].reshape(128, Jn, 128).transpose(1, 0, 2) \
                            .reshape(Jn * 128, 128)
        acc_e = acc[:, OCN:].reshape(128, Je, 256).transpose(1, 0, 2) \
                            .reshape(Je * 128, 256)
        # uniform-attention rows for masked queries (exact host compute)
        va = np.concatenate([nodes[b] @ Wv + bv, edges[b] @ Wev + bev], axis=0)
        mean_va = va.mean(axis=0)
        node_out[b] = (mean_va @ Wo + bo)[None, :]
        edge_out[b] = (mean_va @ Weo + beo)[None, :]
        ln, le = len(idx_n[b]), len(idx_e[b])
        if COMPACT:
            if ln:
                node_out[b][idx_n[b]] = acc_n[:ln] + bo
            if le:
                edge_out[b][idx_e[b]] = acc_e[:le] + beo
        else:
            vn = mask[b, :N]
            ve = mask[b, N:]
            node_out[b][vn] = (acc_n[:N] + bo)[vn]
            edge_out[b][ve] = (acc_e[:E] + beo)[ve]
    return node_out, edge_out


# revision 28
# speedup vs baseline: 1.2228x; 1.2228x over previous
"""Trainium2 Bass kernel for joint node+edge graph attention.

Problem: nn_Attention_71975061946902
  B=2, N=1024 nodes (dim 128), E=2048 edges (dim 256), L=N+E=3072,
  HEADS=8, DHEAD=64, INNER=512. Full attention over the joint sequence
  with a pair mask (mask_i & mask_j), per-type in/out projections.

Sharding: 16 (batch, head) slices -> 8 cores; each core takes one batch
and two adjacent heads. Host compacts the sequence to mask-valid rows
(~half), transposes inputs to contraction-major layout, slices weights
per head pair, and folds SCALE into Wq/bq (exact: SCALE = 2^-3).

Device: scores are computed transposed (ST[j,i]) so the pair mask is
applied for free inside the softmax exp via per-partition scale/bias on
the Activation instruction: exp(m_j * S + (1-m_j)*(-30000)). No row-max
subtraction is needed (|scores| < ~4). A ones column appended to V makes
the same PV matmul produce the softmax denominator. Masked query rows
(uniform attention over all keys) are reconstructed exactly on the host
and scattered into the output. All inputs arrive in two packed blobs
(one DMA each); outputs leave in one packed blob (two DMAs).
"""

import sys

for _p in ("/opt/trn_rl_repo",):
    if _p not in sys.path:
        sys.path.insert(0, _p)

import numpy as np

B = 2
N = 1024
E = 2048
DIM = 128
EDGE_DIM = 256
HEADS = 8
DHEAD = 64
INNER = HEADS * DHEAD
SCALE = DHEAD ** -0.5  # 0.125, exact power of two
NEGC = -30000.0
L = N + E

COMPACT = True  # compact the sequence to mask-valid rows before launch
FPR_ENABLE = True  # float32r matmuls (4x PE throughput, ~1e-4 rel err)

_NC_CACHE: dict = {}
LAST_RES = None  # BassKernelResults of the most recent launch (for test.py)
LAST_WALL_NS = None


def _wpack_layout(Jt):
    """Column layout of the packed weight/bias/mask blob [128, WC]."""
    off = {}
    c = 0
    for name, w in (("wq_n", 128), ("wk_n", 128), ("wv_n", 128),
                    ("wq_e0", 128), ("wq_e1", 128),
                    ("wk_e0", 128), ("wk_e1", 128),
                    ("wv_e0", 128), ("wv_e1", 128),
                    ("wo", 128), ("weo", 256),
                    ("bq", 1), ("bk", 1), ("beq", 1), ("bek", 1),
                    ("mT", Jt), ("negC", Jt),
                    ("bvr", 128), ("bevr", 128)):
        off[name] = (c, c + w)
        c += w
    return off, c


def build_nc(Jn: int, Je: int):
    """Build the SPMD Bass program for one core: one batch, two heads."""
    from contextlib import ExitStack

    import concourse.tile as tile
    from concourse import bacc, mybir
    from concourse.bass import MemorySpace, ts

    FP = mybir.dt.float32
    FPR = mybir.dt.float32r if FPR_ENABLE else mybir.dt.float32
    ACT = mybir.ActivationFunctionType

    Nn, Ne = 128 * Jn, 128 * Je
    Jt = Jn + Je
    Lc = 128 * Jt
    WOFF, WC = _wpack_layout(Jt)
    XC = Nn + 2 * Ne
    OCN, OC = 128 * Jn, 128 * Jn + 256 * Je

    nc = bacc.Bacc("TRN2", target_bir_lowering=False)

    xpack = nc.dram_tensor("xpack", [DIM, XC], FPR, kind="ExternalInput")
    wpack = nc.dram_tensor("wpack", [128, WC], FPR, kind="ExternalInput")
    opack = nc.dram_tensor("opack", [128, OC], FP, kind="ExternalOutput")

    with tile.TileContext(nc) as tc, ExitStack() as ctx:
        singles = ctx.enter_context(tc.tile_pool(name="singles", bufs=1))

        s_x = singles.tile([DIM, XC], FPR)
        nchunks = min(4, max(1, XC // 512))
        xb = (XC + nchunks - 1) // nchunks
        for c0 in range(0, XC, xb):
            c1 = min(XC, c0 + xb)
            nc.sync.dma_start(s_x[:, c0:c1], xpack[:, c0:c1])
        s_w = singles.tile([128, WC], FPR)
        wh = WC // 2
        nc.sync.dma_start(s_w[:, :wh], wpack[:, :wh])
        nc.sync.dma_start(s_w[:, wh:], wpack[:, wh:])

        def W(name):
            a, b = WOFF[name]
            return s_w[:, a:b]

        def Wf(name):
            return W(name).bitcast(FP)

        x_n = s_x[:, 0:Nn]
        x_e = [s_x[:, Nn + c * Ne:Nn + (c + 1) * Ne] for c in range(2)]

        # 512-wide segments of the compacted axis (node region then edges)
        segs = []
        for base, nblk in ((0, Nn), (Nn, Ne)):
            for blk0 in range(0, nblk, 512):
                segs.append((base + blk0, min(512, nblk - blk0)))
        jmap = []  # jtile t -> (segment index, column offset)
        for t in range(Jt):
            j0 = 128 * t
            for k, (sb, sw) in enumerate(segs):
                if sb <= j0 < sb + sw:
                    jmap.append((k, j0 - sb))
                    break

        # per-segment Q/K tiles and per-jtile V tiles: fine-grained deps let
        # attention start as soon as its own slices are projected
        q_blks = [singles.tile([128, sw], FPR, name=f"qb{k}")
                  for k, (sb, sw) in enumerate(segs)]
        k_blks = [singles.tile([128, sw], FPR, name=f"kb{k}")
                  for k, (sb, sw) in enumerate(segs)]
        v_t = [singles.tile([128, 2, 65], FPR, name=f"v{t}")
               for t in range(Jt)]
        s_oT = singles.tile([128, Lc], FPR)
        s_out = singles.tile([128, OC], FP)
        s_bvb = singles.tile([128, 128], FPR)
        s_bevb = singles.tile([128, 128], FPR)

        for t in range(Jt):
            nc.vector.memset(v_t[t][:, :, 64:65].bitcast(FP), 1.0)
        a0, _ = WOFF["bvr"]
        nc.gpsimd.partition_broadcast(s_bvb[:, :], s_w[0:1, a0:a0 + 128],
                                      channels=128)
        a0, _ = WOFF["bevr"]
        nc.gpsimd.partition_broadcast(s_bevb[:, :], s_w[0:1, a0:a0 + 128],
                                      channels=128)

        # ---- projections ----
        with tc.tile_pool(name="pps", bufs=3, space=MemorySpace.PSUM) as pps:
            for k, (sb, sw) in enumerate(segs):
                is_node = sb < Nn
                if is_node:
                    xs = [x_n[:, sb:sb + sw]]
                    wq_t, wk_t = [W("wq_n")], [W("wk_n")]
                    bq_t, bk_t = Wf("bq"), Wf("bk")
                else:
                    xs = [xe[:, sb - Nn:sb - Nn + sw] for xe in x_e]
                    wq_t = [W("wq_e0"), W("wq_e1")]
                    wk_t = [W("wk_e0"), W("wk_e1")]
                    bq_t, bk_t = Wf("beq"), Wf("bek")
                nchunk = len(xs)
                for wt, bias_t, dst in ((wk_t, bk_t, k_blks[k]),
                                        (wq_t, bq_t, q_blks[k])):
                    ps = pps.tile([128, 512], FP, tag="proj")
                    for c in range(nchunk):
                        nc.tensor.matmul(ps[:, :sw], (wt[c]), (xs[c]),
                                         start=(c == 0),
                                         stop=(c == nchunk - 1))
                    nc.vector.tensor_scalar_add(dst[:, :sw], ps[:, :sw],
                                                bias_t)

            for t in range(Jt):
                ps = pps.tile([128, 128], FP, tag="projv")
                if t < Jn:
                    nc.tensor.matmul(ps[:, :], (x_n[:, ts(t, 128)]),
                                     (W("wv_n")), start=True, stop=True)
                else:
                    te = t - Jn
                    nc.tensor.matmul(ps[:, :], (x_e[0][:, ts(te, 128)]),
                                     (W("wv_e0")), start=True, stop=False)
                    nc.tensor.matmul(ps[:, :], (x_e[1][:, ts(te, 128)]),
                                     (W("wv_e1")), start=False, stop=True)
                vb = s_bvb if t < Jn else s_bevb
                nc.vector.tensor_add(
                    v_t[t][:, :, 0:64],
                    ps.rearrange("p (h d) -> p h d", h=2),
                    vb.rearrange("p (h d) -> p h d", h=2),
                )

        # ---- attention ----
        s_mT, s_negC = Wf("mT"), Wf("negC")
        with tc.tile_pool(name="stps", bufs=2, space=MemorySpace.PSUM) as stps, \
                tc.tile_pool(name="pvps", bufs=2, space=MemorySpace.PSUM) as pvps, \
                tc.tile_pool(name="ppool", bufs=4) as ppool, \
                tc.tile_pool(name="npool", bufs=4) as npool:
            for kq, (i0, w) in enumerate(segs):
                po = [pvps.tile([65, 512], FP, tag="pv0", name="po0"),
                      pvps.tile([65, 512], FP, tag="pv1", name="po1")]
                for t in range(Jt):
                    kk, joff = jmap[t]
                    st = stps.tile([128, 2, 512], FP, tag="st")
                    for h in range(2):
                        nc.tensor.matmul(st[:, h, :w],
                                         (k_blks[kk][64 * h:64 * h + 64,
                                                     joff:joff + 128]),
                                         (q_blks[kq][64 * h:64 * h + 64, :w]),
                                         start=True, stop=True)
                    pt = ppool.tile([128, 2, 512], FPR, tag="p")
                    # exp with fused pair mask: exp(m_j * S + (1-m_j)*NEGC)
                    nc.scalar.activation(pt[:, :, :w], st[:, :, :w], ACT.Exp,
                                         bias=s_negC[:, t:t + 1],
                                         scale=s_mT[:, t:t + 1])
                    for h in range(2):
                        nc.tensor.matmul(po[h][:, :w], (v_t[t][:, h, :]),
                                         (pt[:, h, :w]),
                                         start=(t == 0), stop=(t == Jt - 1))
                # normalize: rows 0:64 are sum(P*v), row 64 is sum(P)
                for h in range(2):
                    dsb = npool.tile([1, 512], FP, tag="dsb")
                    nc.vector.reciprocal(dsb[:, :w], po[h][64:65, :w])
                    rb = npool.tile([64, 512], FP, tag="rb")
                    nc.gpsimd.partition_broadcast(rb[:, :w], dsb[0:1, :w],
                                                  channels=64)
                    nc.vector.tensor_mul(s_oT[64 * h:64 * h + 64, i0:i0 + w],
                                         po[h][0:64, :w], rb[:, :w])

        # ---- output projections ----
        with tc.tile_pool(name="ops", bufs=2, space=MemorySpace.PSUM) as ops:
            for t in range(Jt):
                if t < Jn:
                    ps = ops.tile([128, DIM], FP, tag="opn")
                    nc.tensor.matmul(ps[:, :], (s_oT[:, ts(t, 128)]),
                                     (W("wo")), start=True, stop=True)
                    nc.vector.tensor_copy(s_out[:, ts(t, 128)], ps[:, :])
                else:
                    te = t - Jn
                    ps = ops.tile([128, EDGE_DIM], FP, tag="ope")
                    nc.tensor.matmul(ps[:, :], (s_oT[:, ts(t, 128)]),
                                     (W("weo")), start=True, stop=True)
                    nc.vector.tensor_copy(s_out[:, OCN + te * 256:
                                                OCN + (te + 1) * 256], ps[:, :])
        nc.sync.dma_start(opack[:, 0:OCN], s_out[:, 0:OCN])
        oh = OCN + (OC - OCN) // 2
        nc.sync.dma_start(opack[:, OCN:oh], s_out[:, OCN:oh])
        nc.sync.dma_start(opack[:, oh:OC], s_out[:, oh:OC])

    nc.compile()
    return nc


def _get_nc(Jn: int, Je: int):
    key = (Jn, Je, FPR_ENABLE)
    if key not in _NC_CACHE:
        _NC_CACHE[key] = build_nc(Jn, Je)
    return _NC_CACHE[key]


def _prep_host(nodes, edges, mask):
    """Per-batch compaction and layout prep."""
    per_b = []
    if COMPACT:
        idx_n = [np.nonzero(mask[b, :N])[0] for b in range(B)]
        idx_e = [np.nonzero(mask[b, N:])[0] for b in range(B)]
    else:
        idx_n = [np.arange(N)] * B
        idx_e = [np.arange(E)] * B
    Jn = max(1, max((len(i) + 127) // 128 for i in idx_n))
    Je = max(1, max((len(i) + 127) // 128 for i in idx_e))
    Nn, Ne = 128 * Jn, 128 * Je
    Jt = Jn + Je
    Lc = Nn + Ne
    for b in range(B):
        ln, le = len(idx_n[b]), len(idx_e[b])
        xp = np.zeros((DIM, Nn + 2 * Ne), np.float32)
        if ln:
            xp[:, :ln] = nodes[b][idx_n[b]].T
        if le:
            eT = edges[b][idx_e[b]].T  # [256, le]
            xp[:, Nn:Nn + le] = eT[:128]
            xp[:, Nn + Ne:Nn + Ne + le] = eT[128:]
        mc = np.zeros(Lc, np.float32)
        if ln:
            mc[:ln] = mask[b, :N][idx_n[b]].astype(np.float32)
        if le:
            mc[Nn:Nn + le] = mask[b, N:][idx_e[b]].astype(np.float32)
        mT = np.ascontiguousarray(mc.reshape(Jt, 128).T)
        negC = np.ascontiguousarray(((1.0 - mc) * NEGC).reshape(Jt, 128).T)
        per_b.append(dict(xpack=np.ascontiguousarray(xp), mT=mT, negC=negC))
    return per_b, idx_n, idx_e, Jn, Je


def _core_weights(inputs, h0):
    """Unpacked per-core weight dict (heads h0, h0+1)."""
    (Wq, bq, Wk, bk, Wv, bv, Weq, beq, Wek, bek, Wev, bev,
     Wo, bo, Weo, beo) = inputs
    sl = slice(DHEAD * h0, DHEAD * h0 + 128)
    sc = np.float32(SCALE)
    m = {}
    m["wq_n"] = Wq[:, sl] * sc
    m["wk_n"] = Wk[:, sl]
    m["wv_n"] = Wv[:, sl]
    m["wq_e"] = (Weq[:, sl] * sc).reshape(2, 128, 128)
    m["wk_e"] = Wek[:, sl].reshape(2, 128, 128)
    m["wv_e"] = Wev[:, sl].reshape(2, 128, 128)
    m["wo"] = Wo[sl, :]
    m["weo"] = Weo[sl, :]
    m["bq2"] = (bq[sl] * sc).reshape(128, 1)
    m["bk2"] = bk[sl].reshape(128, 1)
    m["bvr"] = bv[sl].reshape(1, 128)
    m["beq2"] = (beq[sl] * sc).reshape(128, 1)
    m["bek2"] = bek[sl].reshape(128, 1)
    m["bevr"] = bev[sl].reshape(1, 128)
    return m


def _pack_w(m, mT, negC, Jt):
    """Pack per-core weights+masks into the wpack blob [128, WC]."""
    WOFF, WC = _wpack_layout(Jt)
    w = np.zeros((128, WC), np.float32)

    def put(name, arr):
        a, b = WOFF[name]
        w[:arr.shape[0], a:b] = arr

    put("wq_n", m["wq_n"]); put("wk_n", m["wk_n"]); put("wv_n", m["wv_n"])
    put("wq_e0", m["wq_e"][0]); put("wq_e1", m["wq_e"][1])
    put("wk_e0", m["wk_e"][0]); put("wk_e1", m["wk_e"][1])
    put("wv_e0", m["wv_e"][0]); put("wv_e1", m["wv_e"][1])
    put("wo", m["wo"]); put("weo", m["weo"])
    put("bq", m["bq2"]); put("bk", m["bk2"])
    put("beq", m["beq2"]); put("bek", m["bek2"])
    put("mT", mT); put("negC", negC)
    put("bvr", m["bvr"]); put("bevr", m["bevr"])
    return np.ascontiguousarray(w)


def kernel(nodes, edges, mask, Wq, bq, Wk, bk, Wv, bv,
           Weq, beq, Wek, bek, Wev, bev, Wo, bo, Weo, beo):
    from concourse.bass_utils import run_bass_kernel_spmd

    nodes = np.asarray(nodes, np.float32)
    edges = np.asarray(edges, np.float32)
    mask = np.asarray(mask).astype(bool)
    ws = tuple(np.asarray(a, np.float32) for a in
               (Wq, bq, Wk, bk, Wv, bv, Weq, beq, Wek, bek, Wev, bev,
                Wo, bo, Weo, beo))
    (Wq, bq, Wk, bk, Wv, bv, Weq, beq, Wek, bek, Wev, bev,
     Wo, bo, Weo, beo) = ws

    per_b, idx_n, idx_e, Jn, Je = _prep_host(nodes, edges, mask)
    Jt = Jn + Je
    nc = _get_nc(Jn, Je)

    in_maps = []
    for core in range(8):
        b = core // 4
        h0 = (core % 4) * 2
        cw = _core_weights(ws, h0)
        in_maps.append({
            "xpack": per_b[b]["xpack"],
            "wpack": _pack_w(cw, per_b[b]["mT"], per_b[b]["negC"], Jt),
        })

    import time as _time

    _t0 = _time.perf_counter()
    res = run_bass_kernel_spmd(nc, in_maps, core_ids=list(range(8)))
    global LAST_RES, LAST_WALL_NS
    LAST_WALL_NS = (_time.perf_counter() - _t0) * 1e9
    LAST_RES = res
    results = res.results

    OCN = 128 * Jn
    node_out = np.empty((B, N, DIM), np.float32)
    edge_out = np.empty((B, E, EDGE_DIM), np.float32)
    for b in range(B):
        acc = results[4 * b]["opack"].copy()
        for k in range(1, 4):
            acc += results[4 * b + k]["opack"]
        # unpack: node tile t at cols [t*128,(t+1)*128) with rows=i, cols=o
        acc_n = acc[:, :OCN].reshape(128, Jn, 128).transpose(1, 0, 2) \
                            .reshape(Jn * 128, 128)
        acc_e = acc[:, OCN:].reshape(128, Je, 256).transpose(1, 0, 2) \
                            .reshape(Je * 128, 256)
        # uniform-attention rows for masked queries (exact host compute)
        va = np.concatenate([nodes[b] @ Wv + bv, edges[b] @ Wev + bev], axis=0)
        mean_va = va.mean(axis=0)
        node_out[b] = (mean_va @ Wo + bo)[None, :]
        edge_out[b] = (mean_va @ Weo + beo)[None, :]
        ln, le = len(idx_n[b]), len(idx_e[b])
        if COMPACT:
            if ln:
                node_out[b][idx_n[b]] = acc_n[:ln] + bo
            if le:
                edge_out[b][idx_e[b]] = acc_e[:le] + beo
        else:
            vn = mask[b, :N]
            ve = mask[b, N:]
            node_out[b][vn] = (acc_n[:N] + bo)[vn]
            edge_out[b][ve] = (acc_e[:E] + beo)[ve]
    return node_out, edge_out
